# revision 17
# baseline (speedup 1.0000x reference)
"""DGDNN forward kernel for Trainium2 (Bass/Tile), data-parallel over batch.

Contract: kernel(**inputs) takes the FULL unsharded inputs (as produced by
setup_inputs) and returns the FULL [B, N, CLS] output. Internally the batch
is split across 8 NeuronCores (4 batches each); weights/T/theta replicated.

Layout strategy (per core): feature-major ("transposed") everywhere so every
matmul contracts over the partition dim:
  h_prime^T = W_raw^T X^T                      [RAW, N]
  z^T       = sum_jc lhsT_jc (Q^T*A^T)_jc      (diffusion)
  scores^T  = k_h(chunk)^T q_h^T               [m, n]  (m on partitions)
  e = exp(s/8) -> ctx^T[h] = sum_mc [v_h | 1]^T e  (ones col => denominator)

Perf notes vs the first working version (644us -> target ~250us):
  * A^T / T^T / Q^T / masked scores / e / v in bf16: halves HBM traffic,
    doubles DVE mask-multiply rate, halves SBUF so two batches pipeline.
  * Softmax denominators are reshaped [1,N]->[128,8] via tiny SBUF->SBUF
    DMAs so the DVE reciprocal runs across 128 lanes (was 5.8us/row on one
    lane -> ~0.1us).
  * PSUM->SBUF evictions moved off the DVE onto the Pool engine; biases
    and relu stay fused in DVE tensor_scalar evictions.
  * Batches processed in interleaved pairs so the PE stays dense (p-state)
    and the act-engine exp stream overlaps the other batch's matmuls.
"""

import numpy as np
from contextlib import ExitStack

import concourse.bass as bass
import concourse.mybir as mybir
import concourse.tile as tile
from concourse import bacc
from concourse.bass_utils import run_bass_kernel_spmd

# ---- problem sizes (hardcoded per spec) ----
B, N, F_IN = 32, 1024, 64
KD = 3                   # expansion_step
H = 2                    # heads
HID = RAW = OUTD = 128
CLS = 2
D1 = D2 = 128
CAT = 256
N_CORES = 8
BL = B // N_CORES        # 4 batches per core
P = 128                  # partitions
NJ = N // P              # 8 node chunks
DH = HID // H            # 64 head dim
HF = 512                 # matmul free-dim chunk (one PSUM bank of f32)
NH = N // HF             # 2 free halves

F32 = mybir.dt.float32
F32R = mybir.dt.float32r
BF16 = mybir.dt.bfloat16
ALU = mybir.AluOpType
ACTF = mybir.ActivationFunctionType
AXX = mybir.AxisListType.X
DEBUG = False


def _f(ap):
    """View a float32r AP as plain f32 (for DVE reads)."""
    return ap.bitcast(F32)


def build_program():
    nc = bacc.Bacc()

    # ---------------- DRAM I/O ----------------
    d_tt = nc.dram_tensor("Tt", [2, KD, N, N], BF16, kind="ExternalInput")
    d_at = nc.dram_tensor("At", [BL, N, N], BF16, kind="ExternalInput")
    d_xn = nc.dram_tensor("Xn", [BL, N, F_IN], BF16, kind="ExternalInput")
    d_xt = nc.dram_tensor("Xt", [BL, F_IN, N], BF16, kind="ExternalInput")
    d_wrawb = nc.dram_tensor("W_rawb", [F_IN, RAW], BF16, kind="ExternalInput")
    d_th = nc.dram_tensor("th6", [1, 2 * KD], F32, kind="ExternalInput")
    d_eye = nc.dram_tensor("ident", [P, P], F32, kind="ExternalInput")

    d_braw = nc.dram_tensor("b_raw", [RAW, 1], F32, kind="ExternalInput")
    d_wd0 = nc.dram_tensor("Wd0", [F_IN, D1], F32, kind="ExternalInput")
    d_bd0 = nc.dram_tensor("bd0", [D1, 1], F32, kind="ExternalInput")
    d_wd1 = nc.dram_tensor("Wd1", [D1, D2], F32, kind="ExternalInput")
    d_bd1 = nc.dram_tensor("bd1", [D2, 1], F32, kind="ExternalInput")
    d_wfin = nc.dram_tensor("W_fin", [OUTD, CLS], F32, kind="ExternalInput")
    d_bfin = nc.dram_tensor("b_fin", [CLS, 1], F32, kind="ExternalInput")
    d_attn = {}
    for l in range(2):
        for nm in ("q", "k", "v"):
            d_attn[f"W{nm}{l}"] = nc.dram_tensor(
                f"W{nm}{l}", [CAT, HID], F32, kind="ExternalInput")
            d_attn[f"b{nm}{l}"] = nc.dram_tensor(
                f"b{nm}{l}", [HID, 1], F32, kind="ExternalInput")
        d_attn[f"Wo{l}"] = nc.dram_tensor(
            f"Wo{l}", [HID, OUTD], F32, kind="ExternalInput")
        d_attn[f"bo{l}"] = nc.dram_tensor(
            f"bo{l}", [OUTD, 1], F32, kind="ExternalInput")
    d_out = nc.dram_tensor("out", [BL, CLS, N], F32, kind="ExternalOutput")
    d_dbg = {}
    if DEBUG:
        for nm in ("hp", "h1T", "h2T", "qT0", "kT0", "ctxs0", "ctxs1",
                   "hp1", "hpF"):
            d_dbg[nm] = nc.dram_tensor(f"dbg_{nm}", [P, N], F32,
                                       kind="ExternalOutput")
        d_dbg["vb0"] = nc.dram_tensor("dbg_vb0", [P, N], BF16,
                                      kind="ExternalOutput")
        d_dbg["h1nm"] = nc.dram_tensor("dbg_h1nm", [P, NJ * D1], BF16,
                                       kind="ExternalOutput")
        d_dbg["dn80"] = nc.dram_tensor("dbg_dn80", [P, 2 * NJ], F32,
                                       kind="ExternalOutput")
        d_dbg["rn80"] = nc.dram_tensor("dbg_rn80", [P, 2 * NJ], F32,
                                       kind="ExternalOutput")

    with tile.TileContext(nc) as tc, ExitStack() as ctx:
        pc = ctx.enter_context(tc.tile_pool(name="const", bufs=1))
        pq = ctx.enter_context(tc.tile_pool(name="qtiles", bufs=1))
        # [*, N]-wide f32 matmul accumulators: 2 banks/slot x 2 = 4 banks
        pmm = ctx.enter_context(tc.tile_pool(name="mm", bufs=2, space="PSUM"))
        # ctx accumulators + v/h transposes: 2 banks/slot x 2 = 4 banks
        pcx = ctx.enter_context(tc.tile_pool(name="ctx", bufs=2, space="PSUM"))

        dma = nc.sync.dma_start

        # ---------------- constants / weights ----------------
        ident = pc.tile([P, P], F32)
        dma(ident[:], d_eye[:])
        identb = pc.tile([P, P], BF16)
        nc.gpsimd.tensor_copy(identb[:], ident[:])
        identr = pc.tile([P, P], F32R)
        dma(identr[:], d_eye[:].bitcast(F32R))
        ones_b = pc.tile([P, NJ, H], BF16)
        nc.vector.memset(ones_b[:], 1.0)

        def wtile(dram, shape, tg):
            t = pc.tile(shape, F32R, tag=tg, name=tg)
            dma(t[:], dram[:].bitcast(F32R))
            return t

        w_raw = pc.tile([F_IN, RAW], BF16, tag="w_raw", name="w_raw")
        dma(w_raw[:], d_wrawb[:])
        wd0 = wtile(d_wd0, [F_IN, D1], "wd0")
        wd1 = wtile(d_wd1, [D1, D2], "wd1")
        wfin = wtile(d_wfin, [OUTD, CLS], "wfin")

        def bias_tile(dram, rows, tg):
            t = pc.tile([rows, 1], F32, tag=f"bias_{tg}", name=f"bias_{tg}")
            dma(t[:], dram[:])
            return t

        b_raw = bias_tile(d_braw, RAW, "raw")
        bd0 = bias_tile(d_bd0, D1, "d0")
        bd1 = bias_tile(d_bd1, D2, "d1")
        bfin = bias_tile(d_bfin, CLS, "fin")

        aw = {}
        for l in range(2):
            for nm in ("q", "k", "v"):
                w = pc.tile([P, 2, HID], F32R, tag=f"w{nm}{l}", name=f"w{nm}{l}")
                for ci in range(2):
                    dma(w[:, ci, :],
                        d_attn[f"W{nm}{l}"][ci * P:(ci + 1) * P, :].bitcast(F32R))
                aw[f"W{nm}{l}"] = w
                aw[f"b{nm}{l}"] = bias_tile(d_attn[f"b{nm}{l}"], HID, f"{nm}{l}")
            w = pc.tile([HID, OUTD], F32R, tag=f"wo{l}", name=f"wo{l}")
            dma(w[:], d_attn[f"Wo{l}"][:].bitcast(F32R))
            aw[f"Wo{l}"] = w
            aw[f"bo{l}"] = bias_tile(d_attn[f"bo{l}"], OUTD, f"o{l}")

        # ---------------- theta softmax + Q^T = sum_k theta_k T_k^T -------
        th_raw = pc.tile([1, 2 * KD], F32)
        dma(th_raw[:], d_th[:])
        th_e = pc.tile([1, 2 * KD], F32)
        nc.scalar.activation(th_e[:], th_raw[:], ACTF.Exp)
        th_soft = pc.tile([1, 2 * KD], F32)
        for l in range(2):
            ssum = pc.tile([1, 1], F32, tag="thsum")
            nc.vector.reduce_sum(ssum[:], th_e[:, l * KD:(l + 1) * KD], axis=AXX)
            srec = pc.tile([1, 1], F32, tag="threc")
            nc.vector.reciprocal(srec[:], ssum[:])
            nc.vector.tensor_scalar(th_soft[:, l * KD:(l + 1) * KD],
                                    th_e[:, l * KD:(l + 1) * KD],
                                    srec[:], None, ALU.mult)
        thb = pc.tile([P, 2 * KD], F32)
        nc.gpsimd.partition_broadcast(thb[:], th_soft[:])
        diag = pc.tile([P, 2 * KD, P], BF16)
        for lk in range(2 * KD):
            nc.vector.tensor_scalar(diag[:, lk, :], ident[:],
                                    thb[:, lk:lk + 1], None, ALU.mult)

        # Q^T stored bf16 [128, 2, NJ, N] (j-chunk-major)
        qt = pq.tile([P, 2, NJ, N], BF16)
        with tc.tile_pool(name="tstream", bufs=2) as pt:
            for l in range(2):
                for jc in range(NJ):
                    t_in = pt.tile([P, KD, N], BF16, tag="tin")
                    for k in range(KD):
                        dma(t_in[:, k, :], d_tt[l, k, jc * P:(jc + 1) * P, :])
                    acc = pmm.tile([P, N], F32, tag="mm")
                    for k in range(KD):
                        for hh in range(NH):
                            sl = slice(hh * HF, (hh + 1) * HF)
                            nc.tensor.matmul(acc[:, sl],
                                             diag[:, l * KD + k, :],
                                             t_in[:, k, sl],
                                             start=(k == 0), stop=(k == KD - 1))
                    nc.scalar.activation(qt[:, l, jc, :], acc[:],
                                         ACTF.Copy)

        # ---------------- pools for the per-batch network ----------------
        pa = ctx.enter_context(tc.tile_pool(name="a", bufs=2))
        px = ctx.enter_context(tc.tile_pool(name="x", bufs=2))
        pb = ctx.enter_context(tc.tile_pool(name="big", bufs=2))
        pe_ = ctx.enter_context(tc.tile_pool(name="e", bufs=3))
        ps_ = ctx.enter_context(tc.tile_pool(name="s", bufs=2))
        pv4 = ctx.enter_context(tc.tile_pool(name="v4", bufs=2))
        prc = ctx.enter_context(tc.tile_pool(name="recip", bufs=1))

        # per-batch state dicts
        S = [dict() for _ in range(BL)]

        def load_batch(b):
            at = pa.tile([P, NJ, N], BF16, tag="at", name=f"at{b}")
            for jc in range(NJ):
                dma(at[:, jc, :], d_at[b, jc * P:(jc + 1) * P, :])
            xn = px.tile([P, NJ, F_IN], BF16, tag="xn", name=f"xn{b}")
            for jc in range(NJ):
                dma(xn[:, jc, :], d_xn[b, jc * P:(jc + 1) * P, :])
            xt = px.tile([F_IN, N], BF16, tag="xt", name=f"xt{b}")
            dma(xt[:], d_xt[b])
            S[b].update(at=at, xn=xn, xt=xt)

        def hp_proj(b):
            # h_prime0^T = W_raw^T X^T + b_raw  (no relu)
            acc = pmm.tile([P, N], F32, tag="mm", name=f"hpacc{b}")
            for hh in range(NH):
                sl = slice(hh * HF, (hh + 1) * HF)
                nc.tensor.matmul(acc[:, sl], w_raw[:], S[b]["xt"][:, sl],
                                 start=True, stop=True)
            hp = pb.tile([P, N], F32R, tag="hp", name=f"hp{b}", bufs=2)
            nc.vector.tensor_scalar(hp[:], acc[:], b_raw[:], None, ALU.add)
            if DEBUG and b == 0:
                dma(d_dbg["hp"][:], _f(hp[:]))
            S[b]["hp"] = hp

        def diffusion(b, l):
            """hT = relu(Wd^T z^T + bd); z^T = sum_jc lhsT_jc (Q^T*A^T)_jc.
            Layer 0 also produces node-major bf16 h for the next layer."""
            at = S[b]["at"]
            if l == 0:
                wd, bd, kdim = wd0, bd0, F_IN
                lhs = lambda jc: S[b]["xn"][:, jc, :]
            else:
                wd, bd, kdim = wd1, bd1, D1
                lhs = lambda jc: S[b]["h1nm"][:, jc, :]
            accz = pmm.tile([kdim, N], F32, tag="mm", name=f"zacc{b}_{l}")
            for jc in range(NJ):
                s_t = ps_.tile([P, N], BF16, tag="s", name=f"st{b}_{l}")
                nc.vector.tensor_tensor(s_t[:], qt[:, l, jc, :],
                                        at[:, jc, :], ALU.mult)
                for hh in range(NH):
                    sl = slice(hh * HF, (hh + 1) * HF)
                    nc.tensor.matmul(accz[:, sl], lhs(jc), s_t[:, sl],
                                     start=(jc == 0), stop=(jc == NJ - 1))
            z = pb.tile([kdim, N], F32R, tag="z", name=f"z{b}_{l}", bufs=2)
            nc.scalar.activation(z[:], accz[:], ACTF.Copy)
            acch = pmm.tile([P, N], F32, tag="mm", name=f"hacc{b}_{l}")
            for hh in range(NH):
                sl = slice(hh * HF, (hh + 1) * HF)
                nc.tensor.matmul(acch[:, sl], wd[:], z[:, sl],
                                 start=True, stop=True)
            hT = pb.tile([P, N], F32R, tag="hT", name=f"h{l + 1}T{b}",
                         bufs=2)
            nc.vector.tensor_scalar(hT[:], acch[:], bd[:], 0.0,
                                    ALU.add, ALU.max)
            S[b][f"h{l + 1}T"] = hT
            if DEBUG and b == 0:
                dma(d_dbg[f"h{l + 1}T"][:], _f(hT[:]))
            if l == 0:
                # PE transposes of h1T -> node-major bf16 h1 (diff1 lhsT)
                h1nm = pb.tile([P, NJ, D1], BF16, tag="h1nm",
                               name=f"h1nm{b}", bufs=2)
                for jc in range(NJ):
                    tp = pcx.tile([P, P], F32R, tag="ctx", name=f"tph{b}_{jc}")
                    nc.tensor.transpose(tp[:], hT[:, jc * P:(jc + 1) * P],
                                        identr[:])
                    nc.scalar.activation(h1nm[:, jc, :], _f(tp[:]),
                                         ACTF.Copy)
                if DEBUG and b == 0:
                    dma(d_dbg["h1nm"][:],
                        h1nm[:].rearrange("p a b -> p (a b)"))
                S[b]["h1nm"] = h1nm

        def attn_proj(b, l):
            """q/k feature-major f32r; v node-major bf16 (via transposes)."""
            hT_a = S[b][f"h{l + 1}T"]
            hpT_a = S[b]["hp"] if l == 0 else S[b]["hp1"]
            xch = (hT_a, hpT_a)

            def proj_fm(nm):
                accp = pmm.tile([P, N], F32, tag="mm", name=f"p{nm}{l}{b}")
                for ci in range(2):
                    for hh in range(NH):
                        sl = slice(hh * HF, (hh + 1) * HF)
                        nc.tensor.matmul(accp[:, sl],
                                         aw[f"W{nm}{l}"][:, ci, :],
                                         xch[ci][:, sl],
                                         start=(ci == 0), stop=(ci == 1))
                return accp

            accq = proj_fm("q")
            qT = pb.tile([P, N], F32R, tag="qT", name=f"qT{l}{b}", bufs=2)
            nc.vector.tensor_scalar(qT[:], accq[:], aw[f"bq{l}"][:],
                                    None, ALU.add)
            acck = proj_fm("k")
            kT = pb.tile([P, N], F32R, tag="kT", name=f"kT{l}{b}", bufs=2)
            nc.vector.tensor_scalar(kT[:], acck[:], aw[f"bk{l}"][:],
                                    None, ALU.add)
            accv = proj_fm("v")
            vb = pb.tile([P, N], BF16, tag="vb", name=f"vb{l}{b}", bufs=2)
            nc.scalar.activation(vb[:], accv[:], ACTF.Identity,
                                 bias=aw[f"bv{l}"][:])

            # v4[:, mc, h, 0:64] = v chunk node-major bf16; col 64 = ones
            v4 = pv4.tile([P, NJ, H, DH + 1], BF16, tag="v4",
                          name=f"v4{l}{b}")
            nc.vector.tensor_copy(v4[:, :, :, DH], ones_b[:])
            for mc in range(NJ):
                tp = pcx.tile([P, P], BF16, tag="ctx", name=f"tpv{l}{b}_{mc}")
                nc.tensor.transpose(tp[:], vb[:, mc * P:(mc + 1) * P],
                                    identb[:])
                nc.vector.tensor_copy(
                    v4[:, mc, :, 0:DH],
                    tp[:].rearrange("p (h d) -> p h d", h=H))
            if DEBUG and b == 0 and l == 0:
                dma(d_dbg["qT0"][:], _f(qT[:]))
                dma(d_dbg["kT0"][:], _f(kT[:]))
                dma(d_dbg["vb0"][:], vb[:])
            S[b]["qT"], S[b]["kT"], S[b]["v4"] = qT, kT, v4

        def attn_score(b, l):
            """scores -> exp -> ctx accumulation (PSUM, f32)."""
            qT, kT, v4 = S[b]["qT"], S[b]["kT"], S[b]["v4"]
            ctxp = [pcx.tile([DH + 1, N], F32, tag="ctx",
                             name=f"ctxp{l}{b}_{hd}") for hd in range(H)]
            for mc in range(NJ):
                for hd in range(H):
                    hsl = slice(hd * DH, (hd + 1) * DH)
                    sc = pmm.tile([P, N], F32, tag="mm",
                                  name=f"sc{l}{b}_{mc}{hd}")
                    for hh in range(NH):
                        fsl = slice(hh * HF, (hh + 1) * HF)
                        nc.tensor.matmul(sc[:, fsl],
                                         kT[hsl, mc * P:(mc + 1) * P],
                                         qT[hsl, fsl],
                                         start=True, stop=True)
                    e_t = pe_.tile([P, N], BF16, tag="e", name=f"e{l}{b}")
                    nc.scalar.activation(e_t[:], sc[:], ACTF.Exp,
                                         scale=float(1.0 / np.sqrt(DH)))
                    for hh in range(NH):
                        fsl = slice(hh * HF, (hh + 1) * HF)
                        nc.tensor.matmul(ctxp[hd][:, fsl],
                                         v4[:, mc, hd, :],
                                         e_t[:, fsl],
                                         start=(mc == 0), stop=(mc == NJ - 1),
                                         skip_group_check=True)
            S[b]["ctxp"] = ctxp

        def attn_norm_out(b, l):
            """normalize ctx by softmax denominator, then Wo projection."""
            ctxp = S[b].pop("ctxp")
            # denominator rows (engine partition offsets must be 0 mod 32)
            dpack = prc.tile([33, N], F32, tag="dpack", name=f"dp{l}{b}")
            for hd in range(H):
                nc.vector.tensor_copy(dpack[32 * hd:32 * hd + 1, :],
                                      ctxp[hd][DH:DH + 1, :])
            # reshape [1,N] rows to [128,8] via DMA so reciprocal uses lanes
            dn8 = prc.tile([P, 2, NJ], F32, tag="dn8", name=f"dn8{l}{b}")
            for hd in range(H):
                dma(dn8[:, hd, :], dpack[32 * hd:32 * hd + 1, :])
            rn8 = prc.tile([P, 2, NJ], F32, tag="rn8", name=f"rn8{l}{b}")
            nc.vector.reciprocal(rn8[:], dn8[:])
            # hardware partition_broadcast only reads partition 0 -> one
            # [1, N] row tile per head
            rrows = [prc.tile([1, N], F32, tag=f"rrow{hd}",
                              name=f"rr{l}{b}_{hd}") for hd in range(H)]
            for hd in range(H):
                dma(rrows[hd][:], rn8[:, hd, :])
            ctxs = pb.tile([P, N], F32R, tag="ctxs", name=f"ctxs{l}{b}",
                           bufs=2)
            for hd in range(H):
                rb = prc.tile([DH, N], F32, tag="rb", name=f"rb{l}{b}_{hd}")
                nc.gpsimd.partition_broadcast(rb[:], rrows[hd][:])
                nc.vector.tensor_tensor(ctxs[hd * DH:(hd + 1) * DH, :],
                                        ctxp[hd][0:DH, :], rb[:], ALU.mult)
            if DEBUG and b == 0:
                if l == 0:
                    dma(d_dbg["dn80"][:],
                        dn8[:].rearrange("p a b -> p (a b)"))
                    dma(d_dbg["rn80"][:],
                        rn8[:].rearrange("p a b -> p (a b)"))
                dma(d_dbg[f"ctxs{l}"][:], _f(ctxs[:]))
            acco = pmm.tile([P, N], F32, tag="mm", name=f"oacc{l}{b}")
            for hh in range(NH):
                sl = slice(hh * HF, (hh + 1) * HF)
                nc.tensor.matmul(acco[:, sl], aw[f"Wo{l}"][:], ctxs[:, sl],
                                 start=True, stop=True)
            ao = pb.tile([P, N], F32R, tag=f"ao{l}", name=f"ao{l}{b}", bufs=2)
            nc.vector.tensor_scalar(ao[:], acco[:], aw[f"bo{l}"][:], 0.0,
                                    ALU.add, ALU.max)
            if l == 0:
                if DEBUG and b == 0:
                    dma(d_dbg["hp1"][:], _f(ao[:]))
                S[b]["hp1"] = ao
            else:
                S[b]["ao1"] = ao

        def final(b):
            hpF = S[b]["ao1"]  # in-place: hpF = hp1 + ao1
            nc.gpsimd.tensor_tensor(hpF[:], _f(S[b]["hp1"]), _f(hpF[:]),
                                    ALU.add)
            if DEBUG and b == 0:
                dma(d_dbg["hpF"][:], _f(hpF[:]))
            accf = pmm.tile([CLS, N], F32, tag="mm", name=f"facc{b}")
            for hh in range(NH):
                sl = slice(hh * HF, (hh + 1) * HF)
                nc.tensor.matmul(accf[:, sl], wfin[:], hpF[:, sl],
                                 start=True, stop=True)
            outT = pb.tile([CLS, N], F32, tag="outT", name=f"outT{b}", bufs=1)
            nc.vector.tensor_scalar(outT[:], accf[:], bfin[:], None, ALU.add)
            dma(d_out[b], outT[:])

        # ---------------- emission schedule (pairs, with seam filling) ----
        load_batch(0)
        load_batch(1)
        for pair in ((0, 1), (2, 3)):
            b0, b1 = pair
            hp_proj(b0)
            hp_proj(b1)
            diffusion(b0, 0)
            diffusion(b1, 0)
            # attention layer 0
            attn_proj(b0, 0)
            attn_proj(b1, 0)
            attn_score(b0, 0)
            attn_norm_out(b0, 0)
            attn_score(b1, 0)
            attn_norm_out(b1, 0)
            # diffusion layer 1 (masks overlap attn tails)
            diffusion(b0, 1)
            if b0 == 0:
                load_batch(2)      # b0's at/xn/xt slots free after diff1(b0)
            diffusion(b1, 1)
            if b0 == 0:
                load_batch(3)
            # attention layer 1
            attn_proj(b0, 1)
            attn_proj(b1, 1)
            attn_score(b0, 1)
            attn_norm_out(b0, 1)
            final(b0)
            attn_score(b1, 1)
            attn_norm_out(b1, 1)
            final(b1)

    nc.finalize()
    return nc


def make_in_maps(inputs):
    """Shard/transform the full input dict into 8 per-core in_maps."""
    import ml_dtypes
    f = np.float32
    bf = ml_dtypes.bfloat16
    X = np.asarray(inputs["X"], f)
    A = np.asarray(inputs["A"], f)
    T = np.asarray(inputs["T"], f)
    common = {
        "Tt": np.ascontiguousarray(T.transpose(0, 1, 3, 2)).astype(bf),
        "th6": np.asarray(inputs["theta"], f).reshape(1, 2 * KD).copy(),
        "ident": np.eye(P, dtype=f),
        "W_rawb": np.asarray(inputs["W_raw"], f).astype(bf),
        "b_raw": np.asarray(inputs["b_raw"], f).reshape(RAW, 1).copy(),
        "Wd0": np.asarray(inputs["Wd0"], f),
        "bd0": np.asarray(inputs["bd0"], f).reshape(D1, 1).copy(),
        "Wd1": np.asarray(inputs["Wd1"], f),
        "bd1": np.asarray(inputs["bd1"], f).reshape(D2, 1).copy(),
        "W_fin": np.asarray(inputs["W_fin"], f),
        "b_fin": np.asarray(inputs["b_fin"], f).reshape(CLS, 1).copy(),
    }
    for l in range(2):
        for nm in ("q", "k", "v"):
            common[f"W{nm}{l}"] = np.asarray(inputs[f"W{nm}{l}"], f)
            common[f"b{nm}{l}"] = np.asarray(
                inputs[f"b{nm}{l}"], f).reshape(HID, 1).copy()
        common[f"Wo{l}"] = np.asarray(inputs[f"Wo{l}"], f)
        common[f"bo{l}"] = np.asarray(
            inputs[f"bo{l}"], f).reshape(OUTD, 1).copy()

    maps = []
    for c in range(N_CORES):
        sl = slice(c * BL, (c + 1) * BL)
        m = dict(common)
        m["Xn"] = np.ascontiguousarray(X[sl]).astype(bf)
        m["Xt"] = np.ascontiguousarray(X[sl].transpose(0, 2, 1)).astype(bf)
        m["At"] = np.ascontiguousarray(A[sl].transpose(0, 2, 1)).astype(bf)
        maps.append(m)
    return maps


_CACHE = {}


def kernel(**inputs):
    if "nc" not in _CACHE:
        _CACHE["nc"] = build_program()
    nc = _CACHE["nc"]
    maps = make_in_maps(inputs)
    res = run_bass_kernel_spmd(nc, maps, list(range(N_CORES)))
    parts = [res.results[c]["out"].transpose(0, 2, 1) for c in range(N_CORES)]
    return np.ascontiguousarray(
        np.concatenate(parts, axis=0), dtype=np.float32)
